# revision 20
# baseline (speedup 1.0000x reference)
"""Trainium2 Bass kernel for nn_CarNet (scatter_memory).

Math (per batch b):
    arg1[f, r]  = sum_l w[l] * x[l, f, r]          (L=64 weighted reduction)
    out[f, t]   = sum_r arg1[f, r] * cw[t, r]      (role remap via car_weight)
    entropy[b]  = -(sum_l p log p) / log(L),  p = w / sum(w)
    max_w[b]    = max_l w[l]

Sharding: data-parallel over batch B=8 across the 8 NeuronCores (one batch
per core); car_weight is replicated (pre-transposed on host so its
contraction dim lands on SBUF partitions).

Per-core implementation:
  - x[b] viewed as (4096, 1024) is streamed in 8 DMA chunks of (128, 4, 1024).
    Row 512*i + 4*p + c of the chunk lands on partition p, slab c -> 16KB
    contiguous per partition per DMA (good descriptor shape).
  - Stage 1 runs on the TensorEngine: for each (chunk i, slab c) a host-built
    stationary matrix W2[p, f] = w[l(row)] * delta(f, row%64) folds the
    l-weighted sum into a 64-partition PSUM accumulator (64 matmuls,
    PSUM-accumulated).  This keeps the reduction off the (slow for fp32)
    VectorEngine and overlaps fully with the DMA stream.
  - arg1 is transposed 128 columns at a time via PE transpose, then stage 2
    is 16 accumulating matmuls against the replicated car_weight^T.
  - entropy/max_w are computed on-chip from the (1, 64) weight row.
"""

import functools

import numpy as np

B, L, F, R, T = 8, 64, 64, 1024, 1024
NCORES = 8
ROWS = L * F  # 4096
N_SLABS = ROWS // 128  # 32
SLABS_PER_CHUNK = 4  # max slabs per DMA chunk (SBUF tile size)
# DMA chunking of the x stream (in 128-row slabs); first chunks are smaller
# so the first matmul can start sooner.
CHUNK_SLABS = [2, 2] + [4] * ((N_SLABS - 4) // 4)

# "f32" (exact) or "bf16" (halves DMA traffic; matmuls accumulate in fp32)
COMPUTE = "bf16"


def _np_compute_dtype():
    if COMPUTE == "bf16":
        import ml_dtypes

        return ml_dtypes.bfloat16
    return np.float32


@functools.lru_cache(maxsize=1)
def _build():
    import concourse.bacc as bacc
    import concourse.mybir as mybir
    import concourse.tile as tile
    fp32 = mybir.dt.float32
    cdt = mybir.dt.bfloat16 if COMPUTE == "bf16" else fp32

    nc = bacc.Bacc("TRN2", target_bir_lowering=False, debug=False,
                   num_devices=NCORES)

    xb = nc.dram_tensor("xb", [ROWS, R], cdt, kind="ExternalInput")
    w2 = nc.dram_tensor("w2", [128, N_SLABS, F], cdt, kind="ExternalInput")
    wrow = nc.dram_tensor("wrow", [1, L], fp32, kind="ExternalInput")
    cwt = nc.dram_tensor("cwt", [R, T], cdt, kind="ExternalInput")
    identin = nc.dram_tensor("identin", [F, F], fp32, kind="ExternalInput")
    out = nc.dram_tensor("out", [F, T], fp32, kind="ExternalOutput")
    stats = nc.dram_tensor("stats", [1, 2], fp32, kind="ExternalOutput")

    with tile.TileContext(nc) as tc:
        with (
            tc.tile_pool(name="singles", bufs=1) as singles,
            tc.tile_pool(name="xpool", bufs=4) as xpool,
            tc.tile_pool(name="tail", bufs=3) as tail,
            tc.tile_pool(name="ps_acc", bufs=1, space="PSUM") as ps_acc,
            tc.tile_pool(name="ps_small", bufs=2, space="PSUM") as ps_small,
        ):
            # ---- constants / small inputs ----
            # Ring discipline: the sync HWDGE ring leads with the first x
            # chunk, the scalar ring leads with w2 (both needed by matmul 0);
            # tiny transfers (wrow, stats) ride the gpsimd SWDGE ring so they
            # never head-block the bulk stream.
            w2_sb = singles.tile([128, N_SLABS, F], cdt)
            nc.scalar.dma_start(out=w2_sb[:], in_=w2.ap())
            wrow_sb = singles.tile([1, L], fp32)
            nc.scalar.dma_start(out=wrow_sb[:], in_=wrow.ap())
            ident = singles.tile([F, F], fp32)
            nc.scalar.dma_start(out=ident[:], in_=identin.ap())

            # ---- PE warm-up: throwaway matmuls release the HAM clock gate
            # (1.2 -> 2.4 GHz) while the first x chunk is still in flight ----
            warm_in = singles.tile([128, 512], cdt)
            nc.vector.memset(warm_in[:], 0.0)
            psum_warm = ps_small.tile([128, 512], fp32)
            N_WARM = 12
            for k in range(N_WARM):
                nc.tensor.matmul(out=psum_warm[:], lhsT=warm_in[:, :128],
                                 rhs=warm_in[:], start=(k == 0),
                                 stop=(k == N_WARM - 1))

            # ---- stage 1: l-weighted reduction, PSUM-accumulated on PE ----
            # First chunks are smaller so the first matmul starts sooner;
            # triggers alternate between the two HWDGE rings (sync/scalar).
            assert sum(CHUNK_SLABS) == N_SLABS
            psum_arg1 = ps_acc.tile([F, R], fp32)
            s0 = 0
            for i, nslab in enumerate(CHUNK_SLABS):
                xt = xpool.tile([128, SLABS_PER_CHUNK, R], cdt,
                                tag="xt", name=f"xt{i}")
                row0 = s0 * 128
                src = xb.ap()[row0:row0 + nslab * 128, :]
                dma_eng = nc.sync if i % 2 == 0 else nc.scalar
                dma_eng.dma_start(
                    out=xt[:, :nslab, :],
                    in_=src.rearrange("(p c) n -> p c n", c=nslab))
                for c in range(nslab):
                    s = s0 + c
                    for n in range(R // 512):
                        nc.tensor.matmul(
                            out=psum_arg1[:, n * 512:(n + 1) * 512],
                            lhsT=w2_sb[:, s, :],
                            rhs=xt[:, c, n * 512:(n + 1) * 512],
                            start=(s == 0),
                            stop=(s == N_SLABS - 1),
                        )
                s0 += nslab

            # ---- replicated car_weight^T, loaded behind the x stream ----
            cwt_sb = singles.tile([128, R // 128, T], cdt)
            for q in range(4):
                src = cwt.ap()[q * 256:(q + 1) * 256, :]
                dma_eng = nc.sync if q % 2 == 0 else nc.scalar
                dma_eng.dma_start(
                    out=cwt_sb[:, 2 * q:2 * q + 2, :],
                    in_=src.rearrange("(c p) n -> p c n", p=128))

            # ---- keep the PE hot while the arg1 transposes drain and
            # car_weight^T finishes streaming ----
            for k in range(8):
                nc.tensor.matmul(out=psum_warm[:], lhsT=warm_in[:, :128],
                                 rhs=warm_in[:], start=(k == 0),
                                 stop=(k == 7))

            # ---- transpose arg1 -> (r, f) chunks ----
            arg1t_sb = singles.tile([128, R // 128, F], cdt)
            for c in range(R // 128):
                a1c = tail.tile([F, 128], fp32)
                nc.vector.tensor_copy(out=a1c[:],
                                      in_=psum_arg1[:, c * 128:(c + 1) * 128])
                pt = ps_small.tile([128, F], fp32)
                nc.tensor.transpose(out=pt[:], in_=a1c[:], identity=ident[:F, :F])
                nc.vector.tensor_copy(out=arg1t_sb[:, c, :], in_=pt[:])

            # ---- stage 2: out = arg1 @ cw^T ----
            psum_out = ps_acc.tile([F, T], fp32)
            for c in range(R // 128):
                for n in range(T // 512):
                    nc.tensor.matmul(
                        out=psum_out[:, n * 512:(n + 1) * 512],
                        lhsT=arg1t_sb[:, c, :],
                        rhs=cwt_sb[:, c, n * 512:(n + 1) * 512],
                        start=(c == 0),
                        stop=(c == R // 128 - 1),
                    )
            out_sb = singles.tile([F, T], fp32)
            nc.scalar.copy(out=out_sb[:, :T // 2], in_=psum_out[:, :T // 2])
            nc.vector.tensor_copy(out=out_sb[:, T // 2:],
                                  in_=psum_out[:, T // 2:])
            nc.sync.dma_start(out=out.ap(), in_=out_sb[:])

            # ---- entropy + max over the (1, 64) weight row.  Emitted last
            # so the Ln ACT-table load / DVE ops never head-block the x
            # triggers, but deps allow it all to run during the DMA ramp ----
            st = singles.tile([1, 16], fp32)  # scratch scalars on partition 0
            lw = singles.tile([1, L], fp32)
            stats_sb = singles.tile([1, 2], fp32)
            s_sum = st[:, 0:1]
            s_max = st[:, 1:2]
            s_swlw = st[:, 2:3]
            s_inv = st[:, 3:4]
            s_ls = st[:, 4:5]
            s_t1 = st[:, 5:6]
            s_t2 = st[:, 6:7]
            nc.vector.reduce_sum(out=s_sum, in_=wrow_sb[:],
                                 axis=mybir.AxisListType.X)
            nc.vector.reduce_max(out=s_max, in_=wrow_sb[:],
                                 axis=mybir.AxisListType.X)
            nc.scalar.activation(out=lw[:], in_=wrow_sb[:],
                                 func=mybir.ActivationFunctionType.Ln)
            # s_swlw = sum(w * ln w) via fused accumulate
            dummy = singles.tile([1, L], fp32)
            nc.vector.scalar_tensor_tensor(
                out=dummy[:], in0=wrow_sb[:], scalar=1.0, in1=lw[:],
                op0=mybir.AluOpType.mult, op1=mybir.AluOpType.mult,
                accum_out=s_swlw)
            nc.vector.reciprocal(out=s_inv, in_=s_sum)
            nc.scalar.activation(out=s_ls, in_=s_sum,
                                 func=mybir.ActivationFunctionType.Ln)
            nc.vector.tensor_tensor(out=s_t1, in0=s_swlw, in1=s_inv,
                                    op=mybir.AluOpType.mult)
            nc.vector.tensor_tensor(out=s_t2, in0=s_ls, in1=s_t1,
                                    op=mybir.AluOpType.subtract)
            nc.vector.tensor_scalar_mul(stats_sb[:, 0:1], s_t2,
                                        float(1.0 / np.log(L)))
            nc.vector.tensor_copy(out=stats_sb[:, 1:2], in_=s_max)
            nc.sync.dma_start(out=stats.ap(), in_=stats_sb[:])

    nc.finalize()
    return nc


def _build_w2(w: np.ndarray) -> np.ndarray:
    """Stationary stage-1 weights: W2h[p, s, f] = w[row//64] * (row%64 == f)
    with row = 128*s0 + nslab*p + c for slab s = s0 + c inside a DMA chunk
    of nslab slabs starting at slab s0 (must mirror the kernel's chunking)."""
    row = np.zeros((128, N_SLABS), dtype=np.int64)
    p = np.arange(128)
    s0 = 0
    for nslab in CHUNK_SLABS:
        for c in range(nslab):
            row[:, s0 + c] = 128 * s0 + nslab * p + c
        s0 += nslab
    w2 = np.zeros((128, N_SLABS, F), dtype=np.float32)
    pp, ss = np.meshgrid(np.arange(128), np.arange(N_SLABS), indexing="ij")
    w2[pp, ss, row % 64] = w[row // 64]
    return w2


def _run(x, arg1_weight, car_weight, trace=False):
    from concourse.bass_utils import run_bass_kernel_spmd

    nc = _build()
    cd = _np_compute_dtype()
    x = np.asarray(x, dtype=np.float32)
    arg1_weight = np.asarray(arg1_weight, dtype=np.float32)
    car_weight = np.asarray(car_weight, dtype=np.float32)
    cwt = np.ascontiguousarray(car_weight.T).astype(cd)
    in_maps = []
    for b in range(B):
        in_maps.append({
            "xb": np.ascontiguousarray(x[b].reshape(ROWS, R)).astype(cd),
            "w2": _build_w2(arg1_weight[b]).astype(cd),
            "wrow": np.ascontiguousarray(arg1_weight[b:b + 1]),
            "identin": np.eye(F, dtype=np.float32),
            "cwt": cwt,
        })
    res = run_bass_kernel_spmd(nc, in_maps, core_ids=list(range(NCORES)),
                               trace=trace)
    outs = res.results
    output = np.stack([r["out"] for r in outs]).astype(np.float32)
    entropy = np.array([r["stats"][0, 0] for r in outs], dtype=np.float32)
    max_w = np.array([r["stats"][0, 1] for r in outs], dtype=np.float32)
    return (output, entropy, max_w), res


def kernel(x, arg1_weight, car_weight):
    (output, entropy, max_w), _ = _run(x, arg1_weight, car_weight)
    return output, entropy, max_w


# revision 21
# speedup vs baseline: 1.0501x; 1.0501x over previous
"""Trainium2 Bass kernel for nn_CarNet (scatter_memory).

Math (per batch b):
    arg1[f, r]  = sum_l w[l] * x[l, f, r]          (L=64 weighted reduction)
    out[f, t]   = sum_r arg1[f, r] * cw[t, r]      (role remap via car_weight)
    entropy[b]  = -(sum_l p log p) / log(L),  p = w / sum(w)
    max_w[b]    = max_l w[l]

Sharding: data-parallel over batch B=8 across the 8 NeuronCores (one batch
per core); car_weight is replicated (pre-transposed on host so its
contraction dim lands on SBUF partitions).

Per-core implementation:
  - x[b] viewed as (4096, 1024) is streamed in 8 DMA chunks of (128, 4, 1024).
    Row 512*i + 4*p + c of the chunk lands on partition p, slab c -> 16KB
    contiguous per partition per DMA (good descriptor shape).
  - Stage 1 runs on the TensorEngine: for each (chunk i, slab c) a host-built
    stationary matrix W2[p, f] = w[l(row)] * delta(f, row%64) folds the
    l-weighted sum into a 64-partition PSUM accumulator (64 matmuls,
    PSUM-accumulated).  This keeps the reduction off the (slow for fp32)
    VectorEngine and overlaps fully with the DMA stream.
  - arg1 is transposed 128 columns at a time via PE transpose, then stage 2
    is 16 accumulating matmuls against the replicated car_weight^T.
  - entropy/max_w are computed on-chip from the (1, 64) weight row.
"""

import functools

import numpy as np

B, L, F, R, T = 8, 64, 64, 1024, 1024
NCORES = 8
ROWS = L * F  # 4096
N_SLABS = ROWS // 128  # 32
SLABS_PER_CHUNK = 4  # max slabs per DMA chunk (SBUF tile size)
# DMA chunking of the x stream (in 128-row slabs); first chunks are smaller
# so the first matmul can start sooner.
CHUNK_SLABS = [2, 2] + [4] * ((N_SLABS - 4) // 4)

# "f32" (exact) or "bf16" (halves DMA traffic; matmuls accumulate in fp32)
COMPUTE = "bf16"


def _np_compute_dtype():
    if COMPUTE == "bf16":
        import ml_dtypes

        return ml_dtypes.bfloat16
    return np.float32


@functools.lru_cache(maxsize=1)
def _build():
    import concourse.bacc as bacc
    import concourse.mybir as mybir
    import concourse.tile as tile
    fp32 = mybir.dt.float32
    cdt = mybir.dt.bfloat16 if COMPUTE == "bf16" else fp32

    nc = bacc.Bacc("TRN2", target_bir_lowering=False, debug=False,
                   num_devices=NCORES)

    xb = nc.dram_tensor("xb", [ROWS, R], cdt, kind="ExternalInput")
    w2 = nc.dram_tensor("w2", [128, N_SLABS, F], cdt, kind="ExternalInput")
    wrow = nc.dram_tensor("wrow", [1, L], fp32, kind="ExternalInput")
    cwt = nc.dram_tensor("cwt", [R, T], cdt, kind="ExternalInput")
    identin = nc.dram_tensor("identin", [F, F], fp32, kind="ExternalInput")
    out = nc.dram_tensor("out", [F, T], fp32, kind="ExternalOutput")
    stats = nc.dram_tensor("stats", [1, 2], fp32, kind="ExternalOutput")

    with tile.TileContext(nc) as tc:
        with (
            tc.tile_pool(name="singles", bufs=1) as singles,
            tc.tile_pool(name="xpool", bufs=4) as xpool,
            tc.tile_pool(name="tail", bufs=3) as tail,
            tc.tile_pool(name="ps_acc", bufs=1, space="PSUM") as ps_acc,
            tc.tile_pool(name="ps_small", bufs=2, space="PSUM") as ps_small,
        ):
            # ---- constants / small inputs ----
            # Ring discipline: the sync HWDGE ring leads with the first x
            # chunk, the scalar ring leads with w2 (both needed by matmul 0);
            # tiny transfers (wrow, stats) ride the gpsimd SWDGE ring so they
            # never head-block the bulk stream.
            w2_sb = singles.tile([128, N_SLABS, F], cdt)
            nc.scalar.dma_start(out=w2_sb[:], in_=w2.ap())
            wrow_sb = singles.tile([1, L], fp32)
            nc.gpsimd.dma_start(out=wrow_sb[:], in_=wrow.ap())
            ident = singles.tile([F, F], fp32)
            nc.gpsimd.dma_start(out=ident[:], in_=identin.ap())

            # ---- PE warm-up: throwaway matmuls release the HAM clock gate
            # (1.2 -> 2.4 GHz) while the first x chunk is still in flight ----
            warm_in = singles.tile([128, 512], cdt)
            nc.vector.memset(warm_in[:], 0.0)
            psum_warm = ps_small.tile([128, 512], fp32)
            N_WARM = 12
            for k in range(N_WARM):
                nc.tensor.matmul(out=psum_warm[:], lhsT=warm_in[:, :128],
                                 rhs=warm_in[:], start=(k == 0),
                                 stop=(k == N_WARM - 1))

            # ---- stage 1: l-weighted reduction, PSUM-accumulated on PE ----
            # First chunks are smaller so the first matmul starts sooner;
            # triggers alternate between the two HWDGE rings (sync/scalar).
            assert sum(CHUNK_SLABS) == N_SLABS
            psum_arg1 = ps_acc.tile([F, R], fp32)
            s0 = 0
            for i, nslab in enumerate(CHUNK_SLABS):
                xt = xpool.tile([128, SLABS_PER_CHUNK, R], cdt,
                                tag="xt", name=f"xt{i}")
                row0 = s0 * 128
                src = xb.ap()[row0:row0 + nslab * 128, :]
                dma_eng = nc.sync if i % 2 == 0 else nc.scalar
                dma_eng.dma_start(
                    out=xt[:, :nslab, :],
                    in_=src.rearrange("(p c) n -> p c n", c=nslab))
                for c in range(nslab):
                    s = s0 + c
                    for n in range(R // 512):
                        nc.tensor.matmul(
                            out=psum_arg1[:, n * 512:(n + 1) * 512],
                            lhsT=w2_sb[:, s, :],
                            rhs=xt[:, c, n * 512:(n + 1) * 512],
                            start=(s == 0),
                            stop=(s == N_SLABS - 1),
                        )
                s0 += nslab

            # ---- replicated car_weight^T, loaded behind the x stream ----
            cwt_sb = singles.tile([128, R // 128, T], cdt)
            for q in range(4):
                src = cwt.ap()[q * 256:(q + 1) * 256, :]
                dma_eng = nc.sync if q % 2 == 0 else nc.scalar
                dma_eng.dma_start(
                    out=cwt_sb[:, 2 * q:2 * q + 2, :],
                    in_=src.rearrange("(c p) n -> p c n", p=128))

            # ---- keep the PE hot while the arg1 transposes drain and
            # car_weight^T finishes streaming ----
            for k in range(8):
                nc.tensor.matmul(out=psum_warm[:], lhsT=warm_in[:, :128],
                                 rhs=warm_in[:], start=(k == 0),
                                 stop=(k == 7))

            # ---- transpose arg1 -> (r, f) chunks ----
            arg1t_sb = singles.tile([128, R // 128, F], cdt)
            for c in range(R // 128):
                a1c = tail.tile([F, 128], fp32)
                nc.vector.tensor_copy(out=a1c[:],
                                      in_=psum_arg1[:, c * 128:(c + 1) * 128])
                pt = ps_small.tile([128, F], fp32)
                nc.tensor.transpose(out=pt[:], in_=a1c[:], identity=ident[:F, :F])
                nc.vector.tensor_copy(out=arg1t_sb[:, c, :], in_=pt[:])

            # ---- stage 2: out = arg1 @ cw^T ----
            psum_out = ps_acc.tile([F, T], fp32)
            for c in range(R // 128):
                for n in range(T // 512):
                    nc.tensor.matmul(
                        out=psum_out[:, n * 512:(n + 1) * 512],
                        lhsT=arg1t_sb[:, c, :],
                        rhs=cwt_sb[:, c, n * 512:(n + 1) * 512],
                        start=(c == 0),
                        stop=(c == R // 128 - 1),
                    )
            out_sb = singles.tile([F, T], fp32)
            nc.scalar.copy(out=out_sb[:, :T // 2], in_=psum_out[:, :T // 2])
            nc.vector.tensor_copy(out=out_sb[:, T // 2:],
                                  in_=psum_out[:, T // 2:])
            nc.sync.dma_start(out=out.ap(), in_=out_sb[:])

            # ---- entropy + max over the (1, 64) weight row.  Emitted last
            # so the Ln ACT-table load / DVE ops never head-block the x
            # triggers, but deps allow it all to run during the DMA ramp ----
            st = singles.tile([1, 16], fp32)  # scratch scalars on partition 0
            lw = singles.tile([1, L], fp32)
            stats_sb = singles.tile([1, 2], fp32)
            s_sum = st[:, 0:1]
            s_max = st[:, 1:2]
            s_swlw = st[:, 2:3]
            s_inv = st[:, 3:4]
            s_ls = st[:, 4:5]
            s_t1 = st[:, 5:6]
            s_t2 = st[:, 6:7]
            nc.vector.reduce_sum(out=s_sum, in_=wrow_sb[:],
                                 axis=mybir.AxisListType.X)
            nc.vector.reduce_max(out=s_max, in_=wrow_sb[:],
                                 axis=mybir.AxisListType.X)
            nc.scalar.activation(out=lw[:], in_=wrow_sb[:],
                                 func=mybir.ActivationFunctionType.Ln)
            # s_swlw = sum(w * ln w) via fused accumulate
            dummy = singles.tile([1, L], fp32)
            nc.vector.scalar_tensor_tensor(
                out=dummy[:], in0=wrow_sb[:], scalar=1.0, in1=lw[:],
                op0=mybir.AluOpType.mult, op1=mybir.AluOpType.mult,
                accum_out=s_swlw)
            nc.vector.reciprocal(out=s_inv, in_=s_sum)
            nc.scalar.activation(out=s_ls, in_=s_sum,
                                 func=mybir.ActivationFunctionType.Ln)
            nc.vector.tensor_tensor(out=s_t1, in0=s_swlw, in1=s_inv,
                                    op=mybir.AluOpType.mult)
            nc.vector.tensor_tensor(out=s_t2, in0=s_ls, in1=s_t1,
                                    op=mybir.AluOpType.subtract)
            nc.vector.tensor_scalar_mul(stats_sb[:, 0:1], s_t2,
                                        float(1.0 / np.log(L)))
            nc.vector.tensor_copy(out=stats_sb[:, 1:2], in_=s_max)
            nc.gpsimd.dma_start(out=stats.ap(), in_=stats_sb[:])

    nc.finalize()
    return nc


def _build_w2(w: np.ndarray) -> np.ndarray:
    """Stationary stage-1 weights: W2h[p, s, f] = w[row//64] * (row%64 == f)
    with row = 128*s0 + nslab*p + c for slab s = s0 + c inside a DMA chunk
    of nslab slabs starting at slab s0 (must mirror the kernel's chunking)."""
    row = np.zeros((128, N_SLABS), dtype=np.int64)
    p = np.arange(128)
    s0 = 0
    for nslab in CHUNK_SLABS:
        for c in range(nslab):
            row[:, s0 + c] = 128 * s0 + nslab * p + c
        s0 += nslab
    w2 = np.zeros((128, N_SLABS, F), dtype=np.float32)
    pp, ss = np.meshgrid(np.arange(128), np.arange(N_SLABS), indexing="ij")
    w2[pp, ss, row % 64] = w[row // 64]
    return w2


def _run(x, arg1_weight, car_weight, trace=False):
    from concourse.bass_utils import run_bass_kernel_spmd

    nc = _build()
    cd = _np_compute_dtype()
    x = np.asarray(x, dtype=np.float32)
    arg1_weight = np.asarray(arg1_weight, dtype=np.float32)
    car_weight = np.asarray(car_weight, dtype=np.float32)
    cwt = np.ascontiguousarray(car_weight.T).astype(cd)
    in_maps = []
    for b in range(B):
        in_maps.append({
            "xb": np.ascontiguousarray(x[b].reshape(ROWS, R)).astype(cd),
            "w2": _build_w2(arg1_weight[b]).astype(cd),
            "wrow": np.ascontiguousarray(arg1_weight[b:b + 1]),
            "identin": np.eye(F, dtype=np.float32),
            "cwt": cwt,
        })
    res = run_bass_kernel_spmd(nc, in_maps, core_ids=list(range(NCORES)),
                               trace=trace)
    outs = res.results
    output = np.stack([r["out"] for r in outs]).astype(np.float32)
    entropy = np.array([r["stats"][0, 0] for r in outs], dtype=np.float32)
    max_w = np.array([r["stats"][0, 1] for r in outs], dtype=np.float32)
    return (output, entropy, max_w), res


def kernel(x, arg1_weight, car_weight):
    (output, entropy, max_w), _ = _run(x, arg1_weight, car_weight)
    return output, entropy, max_w


# revision 22
# speedup vs baseline: 1.0626x; 1.0119x over previous
"""Trainium2 Bass kernel for nn_CarNet (scatter_memory).

Math (per batch b):
    arg1[f, r]  = sum_l w[l] * x[l, f, r]          (L=64 weighted reduction)
    out[f, t]   = sum_r arg1[f, r] * cw[t, r]      (role remap via car_weight)
    entropy[b]  = -(sum_l p log p) / log(L),  p = w / sum(w)
    max_w[b]    = max_l w[l]

Sharding: data-parallel over batch B=8 across the 8 NeuronCores (one batch
per core); car_weight is replicated (pre-transposed on host so its
contraction dim lands on SBUF partitions).

Per-core implementation:
  - x[b] viewed as (4096, 1024) is streamed in 8 DMA chunks of (128, 4, 1024).
    Row 512*i + 4*p + c of the chunk lands on partition p, slab c -> 16KB
    contiguous per partition per DMA (good descriptor shape).
  - Stage 1 runs on the TensorEngine: for each (chunk i, slab c) a host-built
    stationary matrix W2[p, f] = w[l(row)] * delta(f, row%64) folds the
    l-weighted sum into a 64-partition PSUM accumulator (64 matmuls,
    PSUM-accumulated).  This keeps the reduction off the (slow for fp32)
    VectorEngine and overlaps fully with the DMA stream.
  - arg1 is transposed 128 columns at a time via PE transpose, then stage 2
    is 16 accumulating matmuls against the replicated car_weight^T.
  - entropy/max_w are computed on-chip from the (1, 64) weight row.
"""

import functools

import numpy as np

B, L, F, R, T = 8, 64, 64, 1024, 1024
NCORES = 8
ROWS = L * F  # 4096
N_SLABS = ROWS // 128  # 32
SLABS_PER_CHUNK = 4  # max slabs per DMA chunk (SBUF tile size)
# DMA chunking of the x stream (in 128-row slabs); first chunks are smaller
# so the first matmul can start sooner.
CHUNK_SLABS = [2, 2] + [4] * ((N_SLABS - 4) // 4)

# "f32" (exact) or "bf16" (halves DMA traffic; matmuls accumulate in fp32)
COMPUTE = "bf16"


def _np_compute_dtype():
    if COMPUTE == "bf16":
        import ml_dtypes

        return ml_dtypes.bfloat16
    return np.float32


@functools.lru_cache(maxsize=1)
def _build():
    import concourse.bacc as bacc
    import concourse.mybir as mybir
    import concourse.tile as tile
    fp32 = mybir.dt.float32
    cdt = mybir.dt.bfloat16 if COMPUTE == "bf16" else fp32

    nc = bacc.Bacc("TRN2", target_bir_lowering=False, debug=False,
                   num_devices=NCORES)

    xb = nc.dram_tensor("xb", [ROWS, R], cdt, kind="ExternalInput")
    w2 = nc.dram_tensor("w2", [128, N_SLABS, F], cdt, kind="ExternalInput")
    wrow = nc.dram_tensor("wrow", [1, L], fp32, kind="ExternalInput")
    cwt = nc.dram_tensor("cwt", [R, T], cdt, kind="ExternalInput")
    identin = nc.dram_tensor("identin", [F, F], fp32, kind="ExternalInput")
    out = nc.dram_tensor("out", [F, T], fp32, kind="ExternalOutput")
    stats = nc.dram_tensor("stats", [1, 2], fp32, kind="ExternalOutput")

    with tile.TileContext(nc) as tc:
        with (
            tc.tile_pool(name="singles", bufs=1) as singles,
            tc.tile_pool(name="xpool", bufs=6) as xpool,
            tc.tile_pool(name="tail", bufs=3) as tail,
            tc.tile_pool(name="ps_acc", bufs=1, space="PSUM") as ps_acc,
            tc.tile_pool(name="ps_small", bufs=2, space="PSUM") as ps_small,
        ):
            # ---- constants / small inputs ----
            # Ring discipline: the sync HWDGE ring leads with the first x
            # chunk, the scalar ring leads with w2 (both needed by matmul 0);
            # tiny transfers (wrow, stats) ride the gpsimd SWDGE ring so they
            # never head-block the bulk stream.
            w2_sb = singles.tile([128, N_SLABS, F], cdt)
            nc.scalar.dma_start(out=w2_sb[:], in_=w2.ap())
            wrow_sb = singles.tile([1, L], fp32)
            nc.gpsimd.dma_start(out=wrow_sb[:], in_=wrow.ap())
            ident = singles.tile([F, F], fp32)
            nc.gpsimd.dma_start(out=ident[:], in_=identin.ap())

            # ---- PE warm-up: throwaway matmuls release the HAM clock gate
            # (1.2 -> 2.4 GHz) while the first x chunk is still in flight ----
            warm_in = singles.tile([128, 512], cdt)
            nc.vector.memset(warm_in[:], 0.0)
            psum_warm = ps_small.tile([128, 512], fp32)
            N_WARM = 12
            for k in range(N_WARM):
                nc.tensor.matmul(out=psum_warm[:], lhsT=warm_in[:, :128],
                                 rhs=warm_in[:], start=(k == 0),
                                 stop=(k == N_WARM - 1))

            # ---- stage 1: l-weighted reduction, PSUM-accumulated on PE ----
            # First chunks are smaller so the first matmul starts sooner;
            # triggers alternate between the two HWDGE rings (sync/scalar).
            assert sum(CHUNK_SLABS) == N_SLABS
            psum_arg1 = ps_acc.tile([F, R], fp32)
            s0 = 0
            for i, nslab in enumerate(CHUNK_SLABS):
                xt = xpool.tile([128, SLABS_PER_CHUNK, R], cdt,
                                tag="xt", name=f"xt{i}")
                row0 = s0 * 128
                src = xb.ap()[row0:row0 + nslab * 128, :]
                dma_eng = nc.sync if i % 2 == 0 else nc.scalar
                dma_eng.dma_start(
                    out=xt[:, :nslab, :],
                    in_=src.rearrange("(p c) n -> p c n", c=nslab))
                for c in range(nslab):
                    s = s0 + c
                    for n in range(R // 512):
                        nc.tensor.matmul(
                            out=psum_arg1[:, n * 512:(n + 1) * 512],
                            lhsT=w2_sb[:, s, :],
                            rhs=xt[:, c, n * 512:(n + 1) * 512],
                            start=(s == 0),
                            stop=(s == N_SLABS - 1),
                        )
                s0 += nslab

            # ---- replicated car_weight^T, loaded behind the x stream ----
            cwt_sb = singles.tile([128, R // 128, T], cdt)
            for q in range(4):
                src = cwt.ap()[q * 256:(q + 1) * 256, :]
                dma_eng = nc.sync if q % 2 == 0 else nc.scalar
                dma_eng.dma_start(
                    out=cwt_sb[:, 2 * q:2 * q + 2, :],
                    in_=src.rearrange("(c p) n -> p c n", p=128))

            # ---- keep the PE hot while the arg1 transposes drain and
            # car_weight^T finishes streaming ----
            for k in range(8):
                nc.tensor.matmul(out=psum_warm[:], lhsT=warm_in[:, :128],
                                 rhs=warm_in[:], start=(k == 0),
                                 stop=(k == 7))

            # ---- transpose arg1 -> (r, f) chunks ----
            arg1t_sb = singles.tile([128, R // 128, F], cdt)
            for c in range(R // 128):
                a1c = tail.tile([F, 128], fp32)
                nc.vector.tensor_copy(out=a1c[:],
                                      in_=psum_arg1[:, c * 128:(c + 1) * 128])
                pt = ps_small.tile([128, F], fp32)
                nc.tensor.transpose(out=pt[:], in_=a1c[:], identity=ident[:F, :F])
                nc.vector.tensor_copy(out=arg1t_sb[:, c, :], in_=pt[:])

            # ---- stage 2: out = arg1 @ cw^T ----
            psum_out = ps_acc.tile([F, T], fp32)
            for c in range(R // 128):
                for n in range(T // 512):
                    nc.tensor.matmul(
                        out=psum_out[:, n * 512:(n + 1) * 512],
                        lhsT=arg1t_sb[:, c, :],
                        rhs=cwt_sb[:, c, n * 512:(n + 1) * 512],
                        start=(c == 0),
                        stop=(c == R // 128 - 1),
                    )
            out_sb = singles.tile([F, T], fp32)
            nc.scalar.copy(out=out_sb[:, :T // 2], in_=psum_out[:, :T // 2])
            nc.vector.tensor_copy(out=out_sb[:, T // 2:],
                                  in_=psum_out[:, T // 2:])
            nc.sync.dma_start(out=out.ap(), in_=out_sb[:])

            # ---- entropy + max over the (1, 64) weight row.  Emitted last
            # so the Ln ACT-table load / DVE ops never head-block the x
            # triggers, but deps allow it all to run during the DMA ramp ----
            st = singles.tile([1, 16], fp32)  # scratch scalars on partition 0
            lw = singles.tile([1, L], fp32)
            stats_sb = singles.tile([1, 2], fp32)
            s_sum = st[:, 0:1]
            s_max = st[:, 1:2]
            s_swlw = st[:, 2:3]
            s_inv = st[:, 3:4]
            s_ls = st[:, 4:5]
            s_t1 = st[:, 5:6]
            s_t2 = st[:, 6:7]
            nc.vector.reduce_sum(out=s_sum, in_=wrow_sb[:],
                                 axis=mybir.AxisListType.X)
            nc.vector.reduce_max(out=s_max, in_=wrow_sb[:],
                                 axis=mybir.AxisListType.X)
            nc.scalar.activation(out=lw[:], in_=wrow_sb[:],
                                 func=mybir.ActivationFunctionType.Ln)
            # s_swlw = sum(w * ln w) via fused accumulate
            dummy = singles.tile([1, L], fp32)
            nc.vector.scalar_tensor_tensor(
                out=dummy[:], in0=wrow_sb[:], scalar=1.0, in1=lw[:],
                op0=mybir.AluOpType.mult, op1=mybir.AluOpType.mult,
                accum_out=s_swlw)
            nc.vector.reciprocal(out=s_inv, in_=s_sum)
            nc.scalar.activation(out=s_ls, in_=s_sum,
                                 func=mybir.ActivationFunctionType.Ln)
            nc.vector.tensor_tensor(out=s_t1, in0=s_swlw, in1=s_inv,
                                    op=mybir.AluOpType.mult)
            nc.vector.tensor_tensor(out=s_t2, in0=s_ls, in1=s_t1,
                                    op=mybir.AluOpType.subtract)
            nc.vector.tensor_scalar_mul(stats_sb[:, 0:1], s_t2,
                                        float(1.0 / np.log(L)))
            nc.vector.tensor_copy(out=stats_sb[:, 1:2], in_=s_max)
            nc.gpsimd.dma_start(out=stats.ap(), in_=stats_sb[:])

    nc.finalize()
    return nc


def _build_w2(w: np.ndarray) -> np.ndarray:
    """Stationary stage-1 weights: W2h[p, s, f] = w[row//64] * (row%64 == f)
    with row = 128*s0 + nslab*p + c for slab s = s0 + c inside a DMA chunk
    of nslab slabs starting at slab s0 (must mirror the kernel's chunking)."""
    row = np.zeros((128, N_SLABS), dtype=np.int64)
    p = np.arange(128)
    s0 = 0
    for nslab in CHUNK_SLABS:
        for c in range(nslab):
            row[:, s0 + c] = 128 * s0 + nslab * p + c
        s0 += nslab
    w2 = np.zeros((128, N_SLABS, F), dtype=np.float32)
    pp, ss = np.meshgrid(np.arange(128), np.arange(N_SLABS), indexing="ij")
    w2[pp, ss, row % 64] = w[row // 64]
    return w2


def _run(x, arg1_weight, car_weight, trace=False):
    from concourse.bass_utils import run_bass_kernel_spmd

    nc = _build()
    cd = _np_compute_dtype()
    x = np.asarray(x, dtype=np.float32)
    arg1_weight = np.asarray(arg1_weight, dtype=np.float32)
    car_weight = np.asarray(car_weight, dtype=np.float32)
    cwt = np.ascontiguousarray(car_weight.T).astype(cd)
    in_maps = []
    for b in range(B):
        in_maps.append({
            "xb": np.ascontiguousarray(x[b].reshape(ROWS, R)).astype(cd),
            "w2": _build_w2(arg1_weight[b]).astype(cd),
            "wrow": np.ascontiguousarray(arg1_weight[b:b + 1]),
            "identin": np.eye(F, dtype=np.float32),
            "cwt": cwt,
        })
    res = run_bass_kernel_spmd(nc, in_maps, core_ids=list(range(NCORES)),
                               trace=trace)
    outs = res.results
    output = np.stack([r["out"] for r in outs]).astype(np.float32)
    entropy = np.array([r["stats"][0, 0] for r in outs], dtype=np.float32)
    max_w = np.array([r["stats"][0, 1] for r in outs], dtype=np.float32)
    return (output, entropy, max_w), res


def kernel(x, arg1_weight, car_weight):
    (output, entropy, max_w), _ = _run(x, arg1_weight, car_weight)
    return output, entropy, max_w


# revision 23
# speedup vs baseline: 1.1001x; 1.0353x over previous
"""Trainium2 Bass kernel for nn_CarNet (scatter_memory).

Math (per batch b):
    arg1[f, r]  = sum_l w[l] * x[l, f, r]          (L=64 weighted reduction)
    out[f, t]   = sum_r arg1[f, r] * cw[t, r]      (role remap via car_weight)
    entropy[b]  = -(sum_l p log p) / log(L),  p = w / sum(w)
    max_w[b]    = max_l w[l]

Sharding: data-parallel over batch B=8 across the 8 NeuronCores (one batch
per core); car_weight is replicated (pre-transposed on host so its
contraction dim lands on SBUF partitions).

Per-core implementation (compute in bf16, accumulation in fp32 PSUM):
  - x[b] viewed as (4096, 1024) rows streams in 9 DMA chunks (2,2,4,4,...
    slabs of 128 rows); chunk rows land as nslab consecutive rows per
    partition (contiguous per-partition reads).  Chunk triggers alternate
    between the two HWDGE rings (sync / scalar engines); small transfers
    (wrow, identity, stats) ride the gpsimd SWDGE ring so they never
    head-block the bulk stream; car_weight^T follows the x stream.
  - Stage 1 runs on the TensorEngine: for each slab a host-built stationary
    matrix W2[p, f] = w[l(row)] * delta(f, row%64) folds the l-weighted sum
    into a 64-partition PSUM accumulator (64 matmuls, PSUM-accumulated).
    This keeps the reduction off the (1x-rate for fp32) VectorEngine and
    overlaps fully with the DMA stream.
  - Throwaway matmuls at kernel start (and before the stage-2 handoff)
    release the PE_HAM clock gate (1.2 -> 2.4 GHz) before real work lands.
  - arg1 is transposed 128 columns at a time via PE transpose, then stage 2
    is 16 accumulating matmuls against the replicated car_weight^T.
  - entropy/max_w are computed on-chip from the (1, 64) weight row.

Measured on trn2 (8 cores, axon): ~51-57 us HW exec, max rel err ~2.9e-3
(vs ~99 us for the all-fp32 variant, rel err 7e-7; set COMPUTE="f32" for
that).  Memory roofline: ~11 MB/core over ~358 GB/s/core ~= 31 us, plus
~7 us start ramp and ~5 us end-of-kernel drain+barrier.
"""

import functools

import numpy as np

B, L, F, R, T = 8, 64, 64, 1024, 1024
NCORES = 8
ROWS = L * F  # 4096
N_SLABS = ROWS // 128  # 32
SLABS_PER_CHUNK = 4  # max slabs per DMA chunk (SBUF tile size)
# DMA chunking of the x stream (in 128-row slabs); first chunks are smaller
# so the first matmul can start sooner.
CHUNK_SLABS = [2, 2] + [4] * ((N_SLABS - 4) // 4)

# "f32" (exact) or "bf16" (halves DMA traffic; matmuls accumulate in fp32)
COMPUTE = "bf16"


def _np_compute_dtype():
    if COMPUTE == "bf16":
        import ml_dtypes

        return ml_dtypes.bfloat16
    return np.float32


@functools.lru_cache(maxsize=1)
def _build():
    import concourse.bacc as bacc
    import concourse.mybir as mybir
    import concourse.tile as tile
    fp32 = mybir.dt.float32
    cdt = mybir.dt.bfloat16 if COMPUTE == "bf16" else fp32

    nc = bacc.Bacc("TRN2", target_bir_lowering=False, debug=False,
                   num_devices=NCORES)

    xb = nc.dram_tensor("xb", [ROWS, R], cdt, kind="ExternalInput")
    w2 = nc.dram_tensor("w2", [128, N_SLABS, F], cdt, kind="ExternalInput")
    wrow = nc.dram_tensor("wrow", [1, L], fp32, kind="ExternalInput")
    cwt = nc.dram_tensor("cwt", [R, T], cdt, kind="ExternalInput")
    identin = nc.dram_tensor("identin", [F, F], fp32, kind="ExternalInput")
    out = nc.dram_tensor("out", [F, T], fp32, kind="ExternalOutput")
    stats = nc.dram_tensor("stats", [1, 2], fp32, kind="ExternalOutput")

    with tile.TileContext(nc) as tc:
        with (
            tc.tile_pool(name="singles", bufs=1) as singles,
            tc.tile_pool(name="xpool", bufs=6) as xpool,
            tc.tile_pool(name="tail", bufs=3) as tail,
            tc.tile_pool(name="ps_acc", bufs=1, space="PSUM") as ps_acc,
            tc.tile_pool(name="ps_small", bufs=2, space="PSUM") as ps_small,
        ):
            # ---- constants / small inputs ----
            # Ring discipline: the sync HWDGE ring leads with the first x
            # chunk, the scalar ring leads with w2 (both needed by matmul 0);
            # tiny transfers (wrow, stats) ride the gpsimd SWDGE ring so they
            # never head-block the bulk stream.
            w2_sb = singles.tile([128, N_SLABS, F], cdt)
            nc.scalar.dma_start(out=w2_sb[:], in_=w2.ap())
            wrow_sb = singles.tile([1, L], fp32)
            nc.gpsimd.dma_start(out=wrow_sb[:], in_=wrow.ap())
            ident = singles.tile([F, F], fp32)
            nc.gpsimd.dma_start(out=ident[:], in_=identin.ap())

            # ---- PE warm-up: throwaway matmuls release the HAM clock gate
            # (1.2 -> 2.4 GHz) while the first x chunk is still in flight ----
            warm_in = singles.tile([128, 512], cdt)
            nc.vector.memset(warm_in[:], 0.0)
            psum_warm = ps_small.tile([128, 512], fp32)
            N_WARM = 12
            for k in range(N_WARM):
                nc.tensor.matmul(out=psum_warm[:], lhsT=warm_in[:, :128],
                                 rhs=warm_in[:], start=(k == 0),
                                 stop=(k == N_WARM - 1))

            # ---- stage 1: l-weighted reduction, PSUM-accumulated on PE ----
            # First chunks are smaller so the first matmul starts sooner;
            # triggers alternate between the two HWDGE rings (sync/scalar).
            assert sum(CHUNK_SLABS) == N_SLABS
            psum_arg1 = ps_acc.tile([F, R], fp32)
            s0 = 0
            for i, nslab in enumerate(CHUNK_SLABS):
                xt = xpool.tile([128, SLABS_PER_CHUNK, R], cdt,
                                tag="xt", name=f"xt{i}")
                row0 = s0 * 128
                src = xb.ap()[row0:row0 + nslab * 128, :]
                dma_eng = nc.sync if i % 2 == 0 else nc.scalar
                dma_eng.dma_start(
                    out=xt[:, :nslab, :],
                    in_=src.rearrange("(p c) n -> p c n", c=nslab))
                for c in range(nslab):
                    s = s0 + c
                    for n in range(R // 512):
                        nc.tensor.matmul(
                            out=psum_arg1[:, n * 512:(n + 1) * 512],
                            lhsT=w2_sb[:, s, :],
                            rhs=xt[:, c, n * 512:(n + 1) * 512],
                            start=(s == 0),
                            stop=(s == N_SLABS - 1),
                        )
                s0 += nslab

            # ---- replicated car_weight^T, loaded behind the x stream ----
            cwt_sb = singles.tile([128, R // 128, T], cdt)
            for q in range(4):
                src = cwt.ap()[q * 256:(q + 1) * 256, :]
                dma_eng = nc.sync if q % 2 == 0 else nc.scalar
                dma_eng.dma_start(
                    out=cwt_sb[:, 2 * q:2 * q + 2, :],
                    in_=src.rearrange("(c p) n -> p c n", p=128))

            # ---- keep the PE hot while the arg1 transposes drain and
            # car_weight^T finishes streaming ----
            for k in range(8):
                nc.tensor.matmul(out=psum_warm[:], lhsT=warm_in[:, :128],
                                 rhs=warm_in[:], start=(k == 0),
                                 stop=(k == 7))

            # ---- transpose arg1 -> (r, f) chunks ----
            arg1t_sb = singles.tile([128, R // 128, F], cdt)
            for c in range(R // 128):
                a1c = tail.tile([F, 128], fp32)
                nc.vector.tensor_copy(out=a1c[:],
                                      in_=psum_arg1[:, c * 128:(c + 1) * 128])
                pt = ps_small.tile([128, F], fp32)
                nc.tensor.transpose(out=pt[:], in_=a1c[:], identity=ident[:F, :F])
                nc.vector.tensor_copy(out=arg1t_sb[:, c, :], in_=pt[:])

            # ---- stage 2: out = arg1 @ cw^T ----
            psum_out = ps_acc.tile([F, T], fp32)
            for c in range(R // 128):
                for n in range(T // 512):
                    nc.tensor.matmul(
                        out=psum_out[:, n * 512:(n + 1) * 512],
                        lhsT=arg1t_sb[:, c, :],
                        rhs=cwt_sb[:, c, n * 512:(n + 1) * 512],
                        start=(c == 0),
                        stop=(c == R // 128 - 1),
                    )
            out_sb = singles.tile([F, T], fp32)
            nc.scalar.copy(out=out_sb[:, :T // 2], in_=psum_out[:, :T // 2])
            nc.vector.tensor_copy(out=out_sb[:, T // 2:],
                                  in_=psum_out[:, T // 2:])
            nc.sync.dma_start(out=out.ap(), in_=out_sb[:])

            # ---- entropy + max over the (1, 64) weight row.  Emitted last
            # so the Ln ACT-table load / DVE ops never head-block the x
            # triggers, but deps allow it all to run during the DMA ramp ----
            st = singles.tile([1, 16], fp32)  # scratch scalars on partition 0
            lw = singles.tile([1, L], fp32)
            stats_sb = singles.tile([1, 2], fp32)
            s_sum = st[:, 0:1]
            s_max = st[:, 1:2]
            s_swlw = st[:, 2:3]
            s_inv = st[:, 3:4]
            s_ls = st[:, 4:5]
            s_t1 = st[:, 5:6]
            s_t2 = st[:, 6:7]
            nc.vector.reduce_sum(out=s_sum, in_=wrow_sb[:],
                                 axis=mybir.AxisListType.X)
            nc.vector.reduce_max(out=s_max, in_=wrow_sb[:],
                                 axis=mybir.AxisListType.X)
            nc.scalar.activation(out=lw[:], in_=wrow_sb[:],
                                 func=mybir.ActivationFunctionType.Ln)
            # s_swlw = sum(w * ln w) via fused accumulate
            dummy = singles.tile([1, L], fp32)
            nc.vector.scalar_tensor_tensor(
                out=dummy[:], in0=wrow_sb[:], scalar=1.0, in1=lw[:],
                op0=mybir.AluOpType.mult, op1=mybir.AluOpType.mult,
                accum_out=s_swlw)
            nc.vector.reciprocal(out=s_inv, in_=s_sum)
            nc.scalar.activation(out=s_ls, in_=s_sum,
                                 func=mybir.ActivationFunctionType.Ln)
            nc.vector.tensor_tensor(out=s_t1, in0=s_swlw, in1=s_inv,
                                    op=mybir.AluOpType.mult)
            nc.vector.tensor_tensor(out=s_t2, in0=s_ls, in1=s_t1,
                                    op=mybir.AluOpType.subtract)
            nc.vector.tensor_scalar_mul(stats_sb[:, 0:1], s_t2,
                                        float(1.0 / np.log(L)))
            nc.vector.tensor_copy(out=stats_sb[:, 1:2], in_=s_max)
            nc.gpsimd.dma_start(out=stats.ap(), in_=stats_sb[:])

    nc.finalize()
    return nc


def _build_w2(w: np.ndarray) -> np.ndarray:
    """Stationary stage-1 weights: W2h[p, s, f] = w[row//64] * (row%64 == f)
    with row = 128*s0 + nslab*p + c for slab s = s0 + c inside a DMA chunk
    of nslab slabs starting at slab s0 (must mirror the kernel's chunking)."""
    row = np.zeros((128, N_SLABS), dtype=np.int64)
    p = np.arange(128)
    s0 = 0
    for nslab in CHUNK_SLABS:
        for c in range(nslab):
            row[:, s0 + c] = 128 * s0 + nslab * p + c
        s0 += nslab
    w2 = np.zeros((128, N_SLABS, F), dtype=np.float32)
    pp, ss = np.meshgrid(np.arange(128), np.arange(N_SLABS), indexing="ij")
    w2[pp, ss, row % 64] = w[row // 64]
    return w2


def _run(x, arg1_weight, car_weight, trace=False):
    from concourse.bass_utils import run_bass_kernel_spmd

    nc = _build()
    cd = _np_compute_dtype()
    x = np.asarray(x, dtype=np.float32)
    arg1_weight = np.asarray(arg1_weight, dtype=np.float32)
    car_weight = np.asarray(car_weight, dtype=np.float32)
    cwt = np.ascontiguousarray(car_weight.T).astype(cd)
    in_maps = []
    for b in range(B):
        in_maps.append({
            "xb": np.ascontiguousarray(x[b].reshape(ROWS, R)).astype(cd),
            "w2": _build_w2(arg1_weight[b]).astype(cd),
            "wrow": np.ascontiguousarray(arg1_weight[b:b + 1]),
            "identin": np.eye(F, dtype=np.float32),
            "cwt": cwt,
        })
    res = run_bass_kernel_spmd(nc, in_maps, core_ids=list(range(NCORES)),
                               trace=trace)
    outs = res.results
    output = np.stack([r["out"] for r in outs]).astype(np.float32)
    entropy = np.array([r["stats"][0, 0] for r in outs], dtype=np.float32)
    max_w = np.array([r["stats"][0, 1] for r in outs], dtype=np.float32)
    return (output, entropy, max_w), res


def kernel(x, arg1_weight, car_weight):
    (output, entropy, max_w), _ = _run(x, arg1_weight, car_weight)
    return output, entropy, max_w


# revision 24
# speedup vs baseline: 1.1390x; 1.0353x over previous
"""Trainium2 Bass kernel for nn_CarNet (scatter_memory).

Math (per batch b):
    arg1[f, r]  = sum_l w[l] * x[l, f, r]          (L=64 weighted reduction)
    out[f, t]   = sum_r arg1[f, r] * cw[t, r]      (role remap via car_weight)
    entropy[b]  = -(sum_l p log p) / log(L),  p = w / sum(w)
    max_w[b]    = max_l w[l]

Sharding: data-parallel over batch B=8 across the 8 NeuronCores (one batch
per core); car_weight is replicated (pre-transposed on host so its
contraction dim lands on SBUF partitions).

Per-core implementation (compute in bf16, accumulation in fp32 PSUM):
  - x[b] viewed as (4096, 1024) rows streams in 9 DMA chunks (2,2,4,4,...
    slabs of 128 rows); chunk rows land as nslab consecutive rows per
    partition (contiguous per-partition reads).  Chunk triggers alternate
    between the two HWDGE rings (sync / scalar engines); small transfers
    (wrow, identity, stats) ride the gpsimd SWDGE ring so they never
    head-block the bulk stream; car_weight^T follows the x stream.
  - Stage 1 runs on the TensorEngine: for each slab a host-built stationary
    matrix W2[p, f] = w[l(row)] * delta(f, row%64) folds the l-weighted sum
    into a 64-partition PSUM accumulator (64 matmuls, PSUM-accumulated).
    This keeps the reduction off the (1x-rate for fp32) VectorEngine and
    overlaps fully with the DMA stream.
  - Throwaway matmuls at kernel start (and before the stage-2 handoff)
    release the PE_HAM clock gate (1.2 -> 2.4 GHz) before real work lands.
  - arg1 is transposed 128 columns at a time via PE transpose, then stage 2
    is 16 accumulating matmuls against the replicated car_weight^T.
  - entropy/max_w are computed on-chip from the (1, 64) weight row.

Measured on trn2 (8 cores, axon): ~51-57 us HW exec, max rel err ~2.9e-3
(vs ~99 us for the all-fp32 variant, rel err 7e-7; set COMPUTE="f32" for
that).  Memory roofline: ~11 MB/core over ~358 GB/s/core ~= 31 us, plus
~7 us start ramp and ~5 us end-of-kernel drain+barrier.
"""

import functools

import numpy as np

B, L, F, R, T = 8, 64, 64, 1024, 1024
NCORES = 8
ROWS = L * F  # 4096
N_SLABS = ROWS // 128  # 32
SLABS_PER_CHUNK = 4  # max slabs per DMA chunk (SBUF tile size)
# DMA chunking of the x stream (in 128-row slabs); first chunks are smaller
# so the first matmul can start sooner.
CHUNK_SLABS = [2, 2] + [4] * ((N_SLABS - 4) // 4)

# "f32" (exact) or "bf16" (halves DMA traffic; matmuls accumulate in fp32)
COMPUTE = "bf16"


def _np_compute_dtype():
    if COMPUTE == "bf16":
        import ml_dtypes

        return ml_dtypes.bfloat16
    return np.float32


@functools.lru_cache(maxsize=1)
def _build():
    import concourse.bacc as bacc
    import concourse.mybir as mybir
    import concourse.tile as tile
    fp32 = mybir.dt.float32
    cdt = mybir.dt.bfloat16 if COMPUTE == "bf16" else fp32

    nc = bacc.Bacc("TRN2", target_bir_lowering=False, debug=False,
                   num_devices=NCORES)

    xb = nc.dram_tensor("xb", [ROWS, R], cdt, kind="ExternalInput")
    w2 = nc.dram_tensor("w2", [128, N_SLABS, F], cdt, kind="ExternalInput")
    wrow = nc.dram_tensor("wrow", [1, L], fp32, kind="ExternalInput")
    cwt = nc.dram_tensor("cwt", [R, T], cdt, kind="ExternalInput")
    identin = nc.dram_tensor("identin", [F, F], fp32, kind="ExternalInput")
    out = nc.dram_tensor("out", [F, T], fp32, kind="ExternalOutput")
    stats = nc.dram_tensor("stats", [1, 2], fp32, kind="ExternalOutput")

    with tile.TileContext(nc) as tc:
        with (
            tc.tile_pool(name="singles", bufs=1) as singles,
            tc.tile_pool(name="xpool", bufs=6) as xpool,
            tc.tile_pool(name="tail", bufs=3) as tail,
            tc.tile_pool(name="ps_acc", bufs=1, space="PSUM") as ps_acc,
            tc.tile_pool(name="ps_small", bufs=2, space="PSUM") as ps_small,
        ):
            # ---- constants / small inputs ----
            # Ring discipline: the sync HWDGE ring leads with the first x
            # chunk, the scalar ring leads with w2 (both needed by matmul 0);
            # tiny transfers (wrow, stats) ride the gpsimd SWDGE ring so they
            # never head-block the bulk stream.
            w2_sb = singles.tile([128, N_SLABS, F], cdt)
            nc.gpsimd.dma_start(out=w2_sb[:], in_=w2.ap())
            wrow_sb = singles.tile([1, L], fp32)
            nc.gpsimd.dma_start(out=wrow_sb[:], in_=wrow.ap())
            ident = singles.tile([F, F], fp32)
            nc.gpsimd.dma_start(out=ident[:], in_=identin.ap())

            # ---- PE warm-up: throwaway matmuls release the HAM clock gate
            # (1.2 -> 2.4 GHz) while the first x chunk is still in flight ----
            warm_in = singles.tile([128, 512], cdt)
            nc.vector.memset(warm_in[:], 0.0)
            psum_warm = ps_small.tile([128, 512], fp32)
            N_WARM = 12
            for k in range(N_WARM):
                nc.tensor.matmul(out=psum_warm[:], lhsT=warm_in[:, :128],
                                 rhs=warm_in[:], start=(k == 0),
                                 stop=(k == N_WARM - 1))

            # ---- stage 1: l-weighted reduction, PSUM-accumulated on PE ----
            # First chunks are smaller so the first matmul starts sooner;
            # triggers alternate between the two HWDGE rings (sync/scalar).
            assert sum(CHUNK_SLABS) == N_SLABS
            psum_arg1 = ps_acc.tile([F, R], fp32)
            s0 = 0
            for i, nslab in enumerate(CHUNK_SLABS):
                xt = xpool.tile([128, SLABS_PER_CHUNK, R], cdt,
                                tag="xt", name=f"xt{i}")
                row0 = s0 * 128
                src = xb.ap()[row0:row0 + nslab * 128, :]
                dma_eng = nc.sync if i % 2 == 0 else nc.scalar
                dma_eng.dma_start(
                    out=xt[:, :nslab, :],
                    in_=src.rearrange("(p c) n -> p c n", c=nslab))
                for c in range(nslab):
                    s = s0 + c
                    for n in range(R // 512):
                        nc.tensor.matmul(
                            out=psum_arg1[:, n * 512:(n + 1) * 512],
                            lhsT=w2_sb[:, s, :],
                            rhs=xt[:, c, n * 512:(n + 1) * 512],
                            start=(s == 0),
                            stop=(s == N_SLABS - 1),
                        )
                s0 += nslab

            # ---- replicated car_weight^T, loaded behind the x stream ----
            cwt_sb = singles.tile([128, R // 128, T], cdt)
            for q in range(4):
                src = cwt.ap()[q * 256:(q + 1) * 256, :]
                dma_eng = nc.sync if q % 2 == 0 else nc.scalar
                dma_eng.dma_start(
                    out=cwt_sb[:, 2 * q:2 * q + 2, :],
                    in_=src.rearrange("(c p) n -> p c n", p=128))

            # ---- keep the PE hot while the arg1 transposes drain and
            # car_weight^T finishes streaming ----
            for k in range(8):
                nc.tensor.matmul(out=psum_warm[:], lhsT=warm_in[:, :128],
                                 rhs=warm_in[:], start=(k == 0),
                                 stop=(k == 7))

            # ---- transpose arg1 -> (r, f) chunks ----
            arg1t_sb = singles.tile([128, R // 128, F], cdt)
            for c in range(R // 128):
                a1c = tail.tile([F, 128], fp32)
                nc.vector.tensor_copy(out=a1c[:],
                                      in_=psum_arg1[:, c * 128:(c + 1) * 128])
                pt = ps_small.tile([128, F], fp32)
                nc.tensor.transpose(out=pt[:], in_=a1c[:], identity=ident[:F, :F])
                nc.vector.tensor_copy(out=arg1t_sb[:, c, :], in_=pt[:])

            # ---- stage 2: out = arg1 @ cw^T ----
            psum_out = ps_acc.tile([F, T], fp32)
            for c in range(R // 128):
                for n in range(T // 512):
                    nc.tensor.matmul(
                        out=psum_out[:, n * 512:(n + 1) * 512],
                        lhsT=arg1t_sb[:, c, :],
                        rhs=cwt_sb[:, c, n * 512:(n + 1) * 512],
                        start=(c == 0),
                        stop=(c == R // 128 - 1),
                    )
            out_sb = singles.tile([F, T], fp32)
            nc.scalar.copy(out=out_sb[:, :T // 2], in_=psum_out[:, :T // 2])
            nc.vector.tensor_copy(out=out_sb[:, T // 2:],
                                  in_=psum_out[:, T // 2:])
            nc.sync.dma_start(out=out.ap(), in_=out_sb[:])

            # ---- entropy + max over the (1, 64) weight row.  Emitted last
            # so the Ln ACT-table load / DVE ops never head-block the x
            # triggers, but deps allow it all to run during the DMA ramp ----
            st = singles.tile([1, 16], fp32)  # scratch scalars on partition 0
            lw = singles.tile([1, L], fp32)
            stats_sb = singles.tile([1, 2], fp32)
            s_sum = st[:, 0:1]
            s_max = st[:, 1:2]
            s_swlw = st[:, 2:3]
            s_inv = st[:, 3:4]
            s_ls = st[:, 4:5]
            s_t1 = st[:, 5:6]
            s_t2 = st[:, 6:7]
            nc.vector.reduce_sum(out=s_sum, in_=wrow_sb[:],
                                 axis=mybir.AxisListType.X)
            nc.vector.reduce_max(out=s_max, in_=wrow_sb[:],
                                 axis=mybir.AxisListType.X)
            nc.scalar.activation(out=lw[:], in_=wrow_sb[:],
                                 func=mybir.ActivationFunctionType.Ln)
            # s_swlw = sum(w * ln w) via fused accumulate
            dummy = singles.tile([1, L], fp32)
            nc.vector.scalar_tensor_tensor(
                out=dummy[:], in0=wrow_sb[:], scalar=1.0, in1=lw[:],
                op0=mybir.AluOpType.mult, op1=mybir.AluOpType.mult,
                accum_out=s_swlw)
            nc.vector.reciprocal(out=s_inv, in_=s_sum)
            nc.scalar.activation(out=s_ls, in_=s_sum,
                                 func=mybir.ActivationFunctionType.Ln)
            nc.vector.tensor_tensor(out=s_t1, in0=s_swlw, in1=s_inv,
                                    op=mybir.AluOpType.mult)
            nc.vector.tensor_tensor(out=s_t2, in0=s_ls, in1=s_t1,
                                    op=mybir.AluOpType.subtract)
            nc.vector.tensor_scalar_mul(stats_sb[:, 0:1], s_t2,
                                        float(1.0 / np.log(L)))
            nc.vector.tensor_copy(out=stats_sb[:, 1:2], in_=s_max)
            nc.gpsimd.dma_start(out=stats.ap(), in_=stats_sb[:])

    nc.finalize()
    return nc


def _build_w2(w: np.ndarray) -> np.ndarray:
    """Stationary stage-1 weights: W2h[p, s, f] = w[row//64] * (row%64 == f)
    with row = 128*s0 + nslab*p + c for slab s = s0 + c inside a DMA chunk
    of nslab slabs starting at slab s0 (must mirror the kernel's chunking)."""
    row = np.zeros((128, N_SLABS), dtype=np.int64)
    p = np.arange(128)
    s0 = 0
    for nslab in CHUNK_SLABS:
        for c in range(nslab):
            row[:, s0 + c] = 128 * s0 + nslab * p + c
        s0 += nslab
    w2 = np.zeros((128, N_SLABS, F), dtype=np.float32)
    pp, ss = np.meshgrid(np.arange(128), np.arange(N_SLABS), indexing="ij")
    w2[pp, ss, row % 64] = w[row // 64]
    return w2


def _run(x, arg1_weight, car_weight, trace=False):
    from concourse.bass_utils import run_bass_kernel_spmd

    nc = _build()
    cd = _np_compute_dtype()
    x = np.asarray(x, dtype=np.float32)
    arg1_weight = np.asarray(arg1_weight, dtype=np.float32)
    car_weight = np.asarray(car_weight, dtype=np.float32)
    cwt = np.ascontiguousarray(car_weight.T).astype(cd)
    in_maps = []
    for b in range(B):
        in_maps.append({
            "xb": np.ascontiguousarray(x[b].reshape(ROWS, R)).astype(cd),
            "w2": _build_w2(arg1_weight[b]).astype(cd),
            "wrow": np.ascontiguousarray(arg1_weight[b:b + 1]),
            "identin": np.eye(F, dtype=np.float32),
            "cwt": cwt,
        })
    res = run_bass_kernel_spmd(nc, in_maps, core_ids=list(range(NCORES)),
                               trace=trace)
    outs = res.results
    output = np.stack([r["out"] for r in outs]).astype(np.float32)
    entropy = np.array([r["stats"][0, 0] for r in outs], dtype=np.float32)
    max_w = np.array([r["stats"][0, 1] for r in outs], dtype=np.float32)
    return (output, entropy, max_w), res


def kernel(x, arg1_weight, car_weight):
    (output, entropy, max_w), _ = _run(x, arg1_weight, car_weight)
    return output, entropy, max_w


# revision 25
# speedup vs baseline: 1.1837x; 1.0393x over previous
"""Trainium2 Bass kernel for nn_CarNet (scatter_memory).

Math (per batch b):
    arg1[f, r]  = sum_l w[l] * x[l, f, r]          (L=64 weighted reduction)
    out[f, t]   = sum_r arg1[f, r] * cw[t, r]      (role remap via car_weight)
    entropy[b]  = -(sum_l p log p) / log(L),  p = w / sum(w)
    max_w[b]    = max_l w[l]

Sharding: data-parallel over batch B=8 across the 8 NeuronCores (one batch
per core); car_weight is replicated (pre-transposed on host so its
contraction dim lands on SBUF partitions).

Per-core implementation (compute in bf16, accumulation in fp32 PSUM):
  - x[b] viewed as (4096, 1024) rows streams in 9 DMA chunks (2,2,4,4,...
    slabs of 128 rows); chunk rows land as nslab consecutive rows per
    partition (contiguous per-partition reads).  Chunk triggers alternate
    between the two HWDGE rings (sync / scalar engines); small transfers
    (wrow, identity, stats) ride the gpsimd SWDGE ring so they never
    head-block the bulk stream; car_weight^T follows the x stream.
  - Stage 1 runs on the TensorEngine: for each slab a host-built stationary
    matrix W2[p, f] = w[l(row)] * delta(f, row%64) folds the l-weighted sum
    into a 64-partition PSUM accumulator (64 matmuls, PSUM-accumulated).
    This keeps the reduction off the (1x-rate for fp32) VectorEngine and
    overlaps fully with the DMA stream.
  - Throwaway matmuls at kernel start (and before the stage-2 handoff)
    release the PE_HAM clock gate (1.2 -> 2.4 GHz) before real work lands.
  - arg1 is transposed 128 columns at a time via PE transpose, then stage 2
    is 16 accumulating matmuls against the replicated car_weight^T.
  - entropy/max_w are computed on-chip from the (1, 64) weight row.

Measured on trn2 (8 cores, axon): ~51-57 us HW exec, max rel err ~2.9e-3
(vs ~99 us for the all-fp32 variant, rel err 7e-7; set COMPUTE="f32" for
that).  Memory roofline: ~11 MB/core over ~358 GB/s/core ~= 31 us, plus
~7 us start ramp and ~5 us end-of-kernel drain+barrier.
"""

import functools

import numpy as np

B, L, F, R, T = 8, 64, 64, 1024, 1024
NCORES = 8
ROWS = L * F  # 4096
N_SLABS = ROWS // 128  # 32
SLABS_PER_CHUNK = 4  # max slabs per DMA chunk (SBUF tile size)
# DMA chunking of the x stream (in 128-row slabs); first chunks are smaller
# so the first matmul can start sooner.
CHUNK_SLABS = [2, 2] + [4] * ((N_SLABS - 4) // 4)

# "f32" (exact) or "bf16" (halves DMA traffic; matmuls accumulate in fp32)
COMPUTE = "bf16"


def _np_compute_dtype():
    if COMPUTE == "bf16":
        import ml_dtypes

        return ml_dtypes.bfloat16
    return np.float32


@functools.lru_cache(maxsize=1)
def _build():
    import concourse.bacc as bacc
    import concourse.mybir as mybir
    import concourse.tile as tile
    fp32 = mybir.dt.float32
    cdt = mybir.dt.bfloat16 if COMPUTE == "bf16" else fp32

    nc = bacc.Bacc("TRN2", target_bir_lowering=False, debug=False,
                   num_devices=NCORES)

    xb = nc.dram_tensor("xb", [ROWS, R], cdt, kind="ExternalInput")
    w2 = nc.dram_tensor("w2", [128, N_SLABS, F], cdt, kind="ExternalInput")
    wrow = nc.dram_tensor("wrow", [1, L], fp32, kind="ExternalInput")
    cwt = nc.dram_tensor("cwt", [R, T], cdt, kind="ExternalInput")
    identin = nc.dram_tensor("identin", [F, F], fp32, kind="ExternalInput")
    out = nc.dram_tensor("out", [F, T], fp32, kind="ExternalOutput")
    stats = nc.dram_tensor("stats", [1, 2], fp32, kind="ExternalOutput")

    with tile.TileContext(nc) as tc:
        with (
            tc.tile_pool(name="singles", bufs=1) as singles,
            tc.tile_pool(name="xpool", bufs=6) as xpool,
            tc.tile_pool(name="tail", bufs=3) as tail,
            tc.tile_pool(name="ps_acc", bufs=1, space="PSUM") as ps_acc,
            tc.tile_pool(name="ps_small", bufs=2, space="PSUM") as ps_small,
        ):
            # ---- constants / small inputs ----
            # Ring discipline: the sync HWDGE ring leads with the first x
            # chunk, the scalar ring leads with w2 (both needed by matmul 0);
            # tiny transfers (wrow, stats) ride the gpsimd SWDGE ring so they
            # never head-block the bulk stream.
            w2_sb = singles.tile([128, N_SLABS, F], cdt)
            nc.scalar.dma_start(out=w2_sb[:], in_=w2.ap())
            wrow_sb = singles.tile([1, L], fp32)
            nc.gpsimd.dma_start(out=wrow_sb[:], in_=wrow.ap())
            ident = singles.tile([F, F], fp32)
            nc.gpsimd.dma_start(out=ident[:], in_=identin.ap())

            # ---- PE warm-up: throwaway matmuls release the HAM clock gate
            # (1.2 -> 2.4 GHz) while the first x chunk is still in flight ----
            warm_in = singles.tile([128, 512], cdt)
            nc.vector.memset(warm_in[:], 0.0)
            psum_warm = ps_small.tile([128, 512], fp32)
            N_WARM = 12
            for k in range(N_WARM):
                nc.tensor.matmul(out=psum_warm[:], lhsT=warm_in[:, :128],
                                 rhs=warm_in[:], start=(k == 0),
                                 stop=(k == N_WARM - 1))

            # ---- stage 1: l-weighted reduction, PSUM-accumulated on PE ----
            # First chunks are smaller so the first matmul starts sooner;
            # triggers alternate between the two HWDGE rings (sync/scalar).
            assert sum(CHUNK_SLABS) == N_SLABS
            psum_arg1 = ps_acc.tile([F, R], fp32)
            s0 = 0
            for i, nslab in enumerate(CHUNK_SLABS):
                xt = xpool.tile([128, SLABS_PER_CHUNK, R], cdt,
                                tag="xt", name=f"xt{i}")
                row0 = s0 * 128
                src = xb.ap()[row0:row0 + nslab * 128, :]
                dma_eng = nc.sync if i % 2 == 0 else nc.scalar
                dma_eng.dma_start(
                    out=xt[:, :nslab, :],
                    in_=src.rearrange("(p c) n -> p c n", c=nslab))
                for c in range(nslab):
                    s = s0 + c
                    for n in range(R // 512):
                        nc.tensor.matmul(
                            out=psum_arg1[:, n * 512:(n + 1) * 512],
                            lhsT=w2_sb[:, s, :],
                            rhs=xt[:, c, n * 512:(n + 1) * 512],
                            start=(s == 0),
                            stop=(s == N_SLABS - 1),
                        )
                s0 += nslab

            # ---- replicated car_weight^T, loaded behind the x stream ----
            cwt_sb = singles.tile([128, R // 128, T], cdt)
            for q in range(4):
                src = cwt.ap()[q * 256:(q + 1) * 256, :]
                dma_eng = nc.sync if q % 2 == 0 else nc.scalar
                dma_eng.dma_start(
                    out=cwt_sb[:, 2 * q:2 * q + 2, :],
                    in_=src.rearrange("(c p) n -> p c n", p=128))

            # ---- keep the PE hot while the arg1 transposes drain and
            # car_weight^T finishes streaming ----
            for k in range(8):
                nc.tensor.matmul(out=psum_warm[:], lhsT=warm_in[:, :128],
                                 rhs=warm_in[:], start=(k == 0),
                                 stop=(k == 7))

            # ---- transpose arg1 -> (r, f) chunks ----
            arg1t_sb = singles.tile([128, R // 128, F], cdt)
            for c in range(R // 128):
                a1c = tail.tile([F, 128], fp32)
                nc.vector.tensor_copy(out=a1c[:],
                                      in_=psum_arg1[:, c * 128:(c + 1) * 128])
                pt = ps_small.tile([128, F], fp32)
                nc.tensor.transpose(out=pt[:], in_=a1c[:], identity=ident[:F, :F])
                nc.vector.tensor_copy(out=arg1t_sb[:, c, :], in_=pt[:])

            # ---- stage 2: out = arg1 @ cw^T ----
            psum_out = ps_acc.tile([F, T], fp32)
            for c in range(R // 128):
                for n in range(T // 512):
                    nc.tensor.matmul(
                        out=psum_out[:, n * 512:(n + 1) * 512],
                        lhsT=arg1t_sb[:, c, :],
                        rhs=cwt_sb[:, c, n * 512:(n + 1) * 512],
                        start=(c == 0),
                        stop=(c == R // 128 - 1),
                    )
            out_sb = singles.tile([F, T], fp32)
            nc.scalar.copy(out=out_sb[:, :T // 2], in_=psum_out[:, :T // 2])
            nc.vector.tensor_copy(out=out_sb[:, T // 2:],
                                  in_=psum_out[:, T // 2:])
            nc.sync.dma_start(out=out.ap(), in_=out_sb[:])

            # ---- entropy + max over the (1, 64) weight row.  Emitted last
            # so the Ln ACT-table load / DVE ops never head-block the x
            # triggers, but deps allow it all to run during the DMA ramp ----
            st = singles.tile([1, 16], fp32)  # scratch scalars on partition 0
            lw = singles.tile([1, L], fp32)
            stats_sb = singles.tile([1, 2], fp32)
            s_sum = st[:, 0:1]
            s_max = st[:, 1:2]
            s_swlw = st[:, 2:3]
            s_inv = st[:, 3:4]
            s_ls = st[:, 4:5]
            s_t1 = st[:, 5:6]
            s_t2 = st[:, 6:7]
            nc.vector.reduce_sum(out=s_sum, in_=wrow_sb[:],
                                 axis=mybir.AxisListType.X)
            nc.vector.reduce_max(out=s_max, in_=wrow_sb[:],
                                 axis=mybir.AxisListType.X)
            nc.scalar.activation(out=lw[:], in_=wrow_sb[:],
                                 func=mybir.ActivationFunctionType.Ln)
            # s_swlw = sum(w * ln w) via fused accumulate
            dummy = singles.tile([1, L], fp32)
            nc.vector.scalar_tensor_tensor(
                out=dummy[:], in0=wrow_sb[:], scalar=1.0, in1=lw[:],
                op0=mybir.AluOpType.mult, op1=mybir.AluOpType.mult,
                accum_out=s_swlw)
            nc.vector.reciprocal(out=s_inv, in_=s_sum)
            nc.scalar.activation(out=s_ls, in_=s_sum,
                                 func=mybir.ActivationFunctionType.Ln)
            nc.vector.tensor_tensor(out=s_t1, in0=s_swlw, in1=s_inv,
                                    op=mybir.AluOpType.mult)
            nc.vector.tensor_tensor(out=s_t2, in0=s_ls, in1=s_t1,
                                    op=mybir.AluOpType.subtract)
            nc.vector.tensor_scalar_mul(stats_sb[:, 0:1], s_t2,
                                        float(1.0 / np.log(L)))
            nc.vector.tensor_copy(out=stats_sb[:, 1:2], in_=s_max)
            nc.gpsimd.dma_start(out=stats.ap(), in_=stats_sb[:])

    nc.finalize()
    return nc


def _build_w2(w: np.ndarray) -> np.ndarray:
    """Stationary stage-1 weights: W2h[p, s, f] = w[row//64] * (row%64 == f)
    with row = 128*s0 + nslab*p + c for slab s = s0 + c inside a DMA chunk
    of nslab slabs starting at slab s0 (must mirror the kernel's chunking)."""
    row = np.zeros((128, N_SLABS), dtype=np.int64)
    p = np.arange(128)
    s0 = 0
    for nslab in CHUNK_SLABS:
        for c in range(nslab):
            row[:, s0 + c] = 128 * s0 + nslab * p + c
        s0 += nslab
    w2 = np.zeros((128, N_SLABS, F), dtype=np.float32)
    pp, ss = np.meshgrid(np.arange(128), np.arange(N_SLABS), indexing="ij")
    w2[pp, ss, row % 64] = w[row // 64]
    return w2


def _run(x, arg1_weight, car_weight, trace=False):
    from concourse.bass_utils import run_bass_kernel_spmd

    nc = _build()
    cd = _np_compute_dtype()
    x = np.asarray(x, dtype=np.float32)
    arg1_weight = np.asarray(arg1_weight, dtype=np.float32)
    car_weight = np.asarray(car_weight, dtype=np.float32)
    cwt = np.ascontiguousarray(car_weight.T).astype(cd)
    in_maps = []
    for b in range(B):
        in_maps.append({
            "xb": np.ascontiguousarray(x[b].reshape(ROWS, R)).astype(cd),
            "w2": _build_w2(arg1_weight[b]).astype(cd),
            "wrow": np.ascontiguousarray(arg1_weight[b:b + 1]),
            "identin": np.eye(F, dtype=np.float32),
            "cwt": cwt,
        })
    res = run_bass_kernel_spmd(nc, in_maps, core_ids=list(range(NCORES)),
                               trace=trace)
    outs = res.results
    output = np.stack([r["out"] for r in outs]).astype(np.float32)
    entropy = np.array([r["stats"][0, 0] for r in outs], dtype=np.float32)
    max_w = np.array([r["stats"][0, 1] for r in outs], dtype=np.float32)
    return (output, entropy, max_w), res


def kernel(x, arg1_weight, car_weight):
    (output, entropy, max_w), _ = _run(x, arg1_weight, car_weight)
    return output, entropy, max_w
